# revision 14
# baseline (speedup 1.0000x reference)
"""AdaptiveTimeDecayAttention Trainium2 kernel (8 NeuronCores, pure data parallel).

Math (per batch row b):
    x = [keys, q, keys*q, keys-q]            [L, 4D]
    h1 = prelu(x @ W1 + b1, a1)              [L, H1]
    h2 = prelu(h1 @ W2 + b2, a2)             [L, H2]
    s  = (h2 @ W3 + b3) * tw;  mask;  attn = softmax(s);  ws = attn @ keys

Device decomposition (bf16 matmuls, fp32 accumulation):
    x @ W1 = keys @ (W1a + W1d) + keys @ (diag(q_b) @ W1c) + q_b @ (W1b - W1d)
    (W1 row blocks a/b/c/d).  The diag-folded weight Wt_b = q_b[:,None]*W1c is a
    per-batch [D, H1] stationary.  The q-only term + b1 becomes a per-partition
    bias applied inside the PReLU activation.
    Layer 2+3 fold: scores = sum_j sign(W3_j) * prelu(h1 @ (W2 * |W3|)_j + b2*|W3|_j, a2)
    evaluated in token-on-partition layout so the j-reduction runs on the free axis.
"""

import os
import sys

for _p in ("/root/.axon_site/_ro/trn_rl_repo", "/opt/trn_rl_repo"):
    if os.path.isdir(_p) and _p not in sys.path:
        sys.path.append(_p)

import numpy as np
import ml_dtypes

import concourse.bass as bass
import concourse.tile as tile
from concourse import bacc, mybir
from concourse.bass_utils import run_bass_kernel_spmd

F32 = mybir.dt.float32
BF16 = mybir.dt.bfloat16
U8 = mybir.dt.uint8
AF = mybir.ActivationFunctionType
ALU = mybir.AluOpType

B, L, D = 2048, 200, 128
H1, H2 = 64, 32
NCORES = 8
CH = 16            # batches per chunk
PAIRS = CH // 2


def build_nc(BL, a1f, a2f, nP, b2_zero, num_devices=NCORES):
    """Build the per-core Bass program for BL local batches."""
    assert BL % CH == 0
    NCHUNK = BL // CH
    GRPB = min(128, BL)          # batches per softmax group
    assert GRPB % CH == 0
    GRPCH = GRPB // CH
    NGRP = BL // GRPB
    nN = H2 - nP

    nc = bacc.Bacc("TRN2", target_bir_lowering=False, debug=False,
                   num_devices=num_devices)

    def din(name, shape, dt):
        return nc.dram_tensor(name, list(shape), dt, kind="ExternalInput").ap()

    keys_d = din("keys", (BL, L, D), F32)
    query_d = din("query", (BL, D), F32)
    mask_d = din("mask", (BL, L), U8)
    wc1_d = din("Wc1", (D, H1), BF16)
    w1c_d = din("W1c", (D, H1), BF16)
    wbd_d = din("Wbd", (D, H1), BF16)
    w2eo_d = din("W2eo", (2 * H1, 2 * H2), BF16)
    b1_d = din("b1", (H1,), F32)
    b2wx_d = din("b2wx", (2 * H2,), F32)
    b3_d = din("b3", (1,), F32)
    tw_d = din("tw", (L,), F32)
    idf_d = din("idf", (128, 128), F32)
    idb_d = din("idb", (128, 128), BF16)

    wsum_d = nc.dram_tensor("wsum", [BL, D], F32, kind="ExternalOutput").ap()
    attn_d = nc.dram_tensor("attn", [BL, L], F32, kind="ExternalOutput").ap()
    sb2d = nc.dram_tensor("sbounce", [BL, L], F32).ap()

    with tile.TileContext(nc) as tc:
        _kernel_body(tc, dict(
            keys=keys_d, query=query_d, mask=mask_d, Wc1=wc1_d, W1c=w1c_d,
            Wbd=wbd_d, W2eo=w2eo_d, b1=b1_d, b2wx=b2wx_d, b3=b3_d, tw=tw_d,
            idf=idf_d, idb=idb_d, wsum=wsum_d, attn=attn_d, sb2d=sb2d,
        ), BL, NCHUNK, GRPB, GRPCH, NGRP, a1f, a2f, nP, nN, b2_zero)

    nc.compile()
    return nc


def _kernel_body(tc, d, BL, NCHUNK, GRPB, GRPCH, NGRP, a1f, a2f, nP, nN, b2_zero):
    nc = tc.nc
    from contextlib import ExitStack
    ctx = ExitStack()
    with ctx:
        singles = ctx.enter_context(tc.tile_pool(name="singles", bufs=1))
        knat1_p = ctx.enter_context(tc.tile_pool(name="knat1", bufs=GRPCH + 2))
        knat2_p = ctx.enter_context(tc.tile_pool(name="knat2", bufs=GRPCH + 2))
        keysT_p = ctx.enter_context(tc.tile_pool(name="keysT", bufs=3))
        h1x_p = ctx.enter_context(tc.tile_pool(name="h1x", bufs=2))
        wt_p = ctx.enter_context(tc.tile_pool(name="wt", bufs=4))
        r_p = ctx.enter_context(tc.tile_pool(name="r", bufs=2))
        red_p = ctx.enter_context(tc.tile_pool(name="red", bufs=4))
        sc_p = ctx.enter_context(tc.tile_pool(name="scores", bufs=2))
        st_p = ctx.enter_context(tc.tile_pool(name="sT", bufs=2))
        soft_p = ctx.enter_context(tc.tile_pool(name="soft", bufs=3))
        small_p = ctx.enter_context(tc.tile_pool(name="small", bufs=6))
        at_p = ctx.enter_context(tc.tile_pool(name="attnT", bufs=2))
        wout_p = ctx.enter_context(tc.tile_pool(name="wout", bufs=2))

        h1ps = ctx.enter_context(tc.tile_pool(name="h1ps", bufs=2, space="PSUM"))
        ups = ctx.enter_context(tc.tile_pool(name="ups", bufs=2, space="PSUM"))
        mps = ctx.enter_context(tc.tile_pool(name="mps", bufs=2, space="PSUM"))
        wps = ctx.enter_context(tc.tile_pool(name="wps", bufs=2, space="PSUM"))

        # ---------------- setup ----------------
        Wc1s = singles.tile([D, H1], BF16)
        nc.sync.dma_start(Wc1s[:], d["Wc1"])
        W1cs = singles.tile([D, H1], BF16)
        nc.sync.dma_start(W1cs[:], d["W1c"])
        Wbds = singles.tile([D, H1], BF16)
        nc.sync.dma_start(Wbds[:], d["Wbd"])
        W2s = singles.tile([2 * H1, 2 * H2], BF16)
        nc.sync.dma_start(W2s[:], d["W2eo"])
        idfs = singles.tile([128, 128], F32)
        nc.sync.dma_start(idfs[:], d["idf"])
        idbs = singles.tile([128, 128], BF16)
        nc.sync.dma_start(idbs[:], d["idb"])
        b1s = singles.tile([H1, 1], F32)
        nc.sync.dma_start(b1s[:], d["b1"])
        b3s = singles.tile([128, 1], F32)
        nc.sync.dma_start(b3s[:], bass.AP(tensor=d["b3"].tensor, offset=d["b3"].offset,
                                          ap=[[0, 128]] + list(d["b3"].ap)))
        tws = singles.tile([128, L], F32)
        nc.sync.dma_start(tws[:], bass.AP(tensor=d["tw"].tensor, offset=d["tw"].offset,
                                          ap=[[0, 128]] + list(d["tw"].ap)))
        b2wxs = singles.tile([128, 2 * H2], F32)
        nc.sync.dma_start(b2wxs[:], bass.AP(tensor=d["b2wx"].tensor, offset=d["b2wx"].offset,
                                            ap=[[0, 128]] + list(d["b2wx"].ap)))
        negbig = singles.tile([128, 1], F32)
        nc.vector.memset(negbig[:], -1e30)

        # q transpose -> qT [D, BL] (f32 + bf16 copies)
        QP = min(128, BL)
        QT = BL // QP
        qnat = singles.tile([QP, QT, D], F32)
        nc.sync.dma_start(qnat[:], d["query"].rearrange("(t p) d -> p t d", p=QP))
        qTps = mps.tile([128, BL], F32, tag="m")
        for t in range(QT):
            nc.tensor.transpose(qTps[:, t * QP:(t + 1) * QP], qnat[:, t, :],
                                idfs[0:QP, 0:QP])
        qTs = singles.tile([D, BL], F32)
        nc.vector.tensor_copy(qTs[:], qTps[:])
        qTb = singles.tile([D, BL], BF16)
        nc.vector.tensor_copy(qTb[:], qTps[:])

        # qW = q @ (W1b - W1d) + b1, stacked per batch-pair:
        # qWstack[0:64, i] = qW[2i] + b1 ; qWstack[64:128, i] = qW[2i+1] + b1
        qWps = mps.tile([H1, BL], F32, tag="m")
        nc.tensor.matmul(qWps[:], lhsT=Wbds[:], rhs=qTb[:])
        qWstack = singles.tile([128, BL // 2], F32)
        qWv = qWps[:].rearrange("h (i two) -> h two i", two=2)
        nc.scalar.activation(qWstack[0:H1, :], qWv[:, 0, :], AF.Identity, bias=b1s[:])
        nc.scalar.activation(qWstack[H1:128, :], qWv[:, 1, :], AF.Identity, bias=b1s[:])

        knat1_tiles = []
        knat2_tiles = []

        for g in range(NGRP):
            # ------------- chunk phase -------------
            for cc in range(GRPCH):
                c = g * GRPCH + cc
                b0 = c * CH
                knat1 = knat1_p.tile([128, CH, D], BF16, tag="knat1")
                nc.gpsimd.dma_start(
                    knat1[:], d["keys"][b0:b0 + CH, 0:128, :].rearrange("b l d -> l b d"))
                knat2 = knat2_p.tile([80, CH, D], BF16, tag="knat2")
                nc.gpsimd.memset(knat2[64:80, :, :], 0.0)
                nc.gpsimd.dma_start(
                    knat2[0:72, :, :],
                    d["keys"][b0:b0 + CH, 128:L, :].rearrange("b l d -> l b d"))
                knat1_tiles.append(knat1)
                knat2_tiles.append(knat2)

                keysT1 = keysT_p.tile([128, CH, 128], BF16, tag="keysT1")
                nc.sync.dma_start(out=keysT1[:], in_=knat1[:], transpose=True)
                keysT2 = keysT_p.tile([128, CH, 80], BF16, tag="keysT2")
                nc.sync.dma_start(out=keysT2[:], in_=knat2[:], transpose=True)

                h1x = h1x_p.tile([128, PAIRS * L], BF16, tag="h1x")
                for p in range(PAIRS):
                    gpair = c * PAIRS + p
                    h1bank = h1ps.tile([128, L], F32, tag="h1bank")
                    for par in range(2):
                        bi = 2 * p + par
                        b = b0 + bi
                        cp = H1 * par
                        Wt = wt_p.tile([D, H1], BF16, tag="wt")
                        nc.gpsimd.tensor_scalar(
                            out=Wt[:], in0=W1cs[:], scalar1=qTs[:, b:b + 1],
                            scalar2=None, op0=ALU.mult)
                        for (rhs_ap, o0, o1) in (
                                (keysT1[:, bi, :], 0, 128),
                                (keysT2[:, bi, 0:72], 128, L)):
                            nc.tensor.matmul(
                                h1bank[cp:cp + H1, o0:o1], lhsT=Wc1s[:],
                                rhs=rhs_ap, start=True, stop=False,
                                tile_position=(0, cp), skip_group_check=True)
                            nc.tensor.matmul(
                                h1bank[cp:cp + H1, o0:o1], lhsT=Wt[:],
                                rhs=rhs_ap, start=False, stop=True,
                                tile_position=(0, cp), skip_group_check=True)
                    nc.scalar.activation(
                        h1x[:, p * L:(p + 1) * L], h1bank[:],
                        AF.Prelu, bias=qWstack[:, gpair:gpair + 1],
                        scale=1.0, alpha=a1f)

                scores = sc_p.tile([128, 32], F32, tag="scores")
                for half in range(2):
                    ubank = ups.tile([128, 512], F32, tag="ubank")
                    for pl in range(4):
                        p = half * 4 + pl
                        for lh in range(2):
                            sl = pl * 2 + lh
                            l0, l1 = (0, 128) if lh == 0 else (128, L)
                            nc.tensor.matmul(
                                ubank[0:l1 - l0, sl * 64:(sl + 1) * 64],
                                lhsT=h1x[:, p * L + l0: p * L + l1],
                                rhs=W2s[:], start=True, stop=True)
                    if not b2_zero:
                        nc.vector.tensor_add(
                            ubank[:], ubank[:],
                            b2wxs[:].rearrange("p j -> p 1 j").to_broadcast((128, 8, 64)))
                    r = r_p.tile([128, 512], BF16, tag="r")
                    nc.scalar.activation(r[:], ubank[:], AF.Prelu,
                                         bias=0.0, scale=1.0, alpha=a2f)
                    rv = r[:].rearrange("p (s j) -> p s j", s=8)
                    scv = scores[:].rearrange("p (lh pq par) -> p pq lh par",
                                              lh=2, par=2)
                    for par in range(2):
                        out_ap = scv[:, half * 4:(half + 1) * 4, :, par]
                        if nP and nN:
                            tP = red_p.tile([128, 8], F32, tag="red")
                            tN = red_p.tile([128, 8], F32, tag="red")
                            nc.vector.tensor_reduce(
                                out=tP[:], in_=rv[:, :, par * 32:par * 32 + nP],
                                axis=mybir.AxisListType.X, op=ALU.add)
                            nc.vector.tensor_reduce(
                                out=tN[:], in_=rv[:, :, par * 32 + nP:par * 32 + 32],
                                axis=mybir.AxisListType.X, op=ALU.add)
                            nc.vector.tensor_tensor(
                                out=out_ap,
                                in0=tP[:].rearrange("p (pl lh) -> p pl lh", lh=2),
                                in1=tN[:].rearrange("p (pl lh) -> p pl lh", lh=2),
                                op=ALU.subtract)
                        else:
                            nc.vector.tensor_reduce(
                                out=out_ap,
                                in_=rv[:, :, par * 32:par * 32 + 32],
                                axis=mybir.AxisListType.X,
                                op=ALU.add, negate=(nN > 0))

                nc.vector.tensor_scalar_add(scores[:], scores[:], b3s[:])
                sTp = mps.tile([32, 128], F32, tag="m")
                nc.tensor.transpose(sTp[:], scores[:], idfs[:])
                sT = st_p.tile([32, 128], F32, tag="sT")
                nc.vector.tensor_copy(sT[:], sTp[:])
                nc.sync.dma_start(d["sb2d"][b0:b0 + CH, 0:128], sT[0:16, :])
                nc.sync.dma_start(d["sb2d"][b0:b0 + CH, 128:L], sT[16:32, 0:72])

            # ------------- group phase -------------
            gb0 = g * GRPB
            ssb = soft_p.tile([GRPB, L], F32, tag="ssb")
            nc.sync.dma_start(ssb[:], d["sb2d"][gb0:gb0 + GRPB, :])
            msb = soft_p.tile([GRPB, L], U8, tag="msb")
            nc.sync.dma_start(msb[:], d["mask"][gb0:gb0 + GRPB, :])
            st = soft_p.tile([GRPB, L], F32, tag="st")
            nc.vector.tensor_mul(st[:], ssb[:], tws[0:GRPB, :])
            sm = soft_p.tile([GRPB, L], F32, tag="sm")
            nc.vector.tensor_copy(sm[:], negbig[0:GRPB, :].to_broadcast((GRPB, L)))
            nc.vector.copy_predicated(sm[:], msb[:], st[:])
            negmax = small_p.tile([GRPB, 1], F32, tag="negmax")
            nc.vector.tensor_reduce(out=negmax[:], in_=sm[:], axis=mybir.AxisListType.X,
                                    op=ALU.max, negate=True)
            psb = soft_p.tile([GRPB, L], F32, tag="psb")
            sumexp = small_p.tile([GRPB, 1], F32, tag="sumexp")
            nc.scalar.activation(psb[:], sm[:], AF.Exp, bias=negmax[:],
                                 scale=1.0, accum_out=sumexp[:])
            rsum = small_p.tile([GRPB, 1], F32, tag="rsum")
            nc.vector.reciprocal(rsum[:], sumexp[:])
            attnf = soft_p.tile([GRPB, L], F32, tag="attnf")
            nc.vector.tensor_scalar_mul(attnf[:], psb[:], rsum[:])
            nc.sync.dma_start(d["attn"][gb0:gb0 + GRPB, :], attnf[:])
            attnb = soft_p.tile([GRPB, L], BF16, tag="attnb")
            nc.vector.tensor_copy(attnb[:], attnf[:])

            aTlop = wps.tile([128, GRPB], BF16, tag="w")
            nc.tensor.transpose(aTlop[:], attnb[:, 0:128], idbs[0:GRPB, 0:GRPB])
            aTlo = at_p.tile([128, GRPB], BF16, tag="aTlo")
            nc.vector.tensor_copy(aTlo[:], aTlop[:])
            aThip = wps.tile([72, GRPB], BF16, tag="w")
            nc.tensor.transpose(aThip[:], attnb[:, 128:L], idbs[0:GRPB, 0:GRPB])
            aThi = at_p.tile([72, GRPB], BF16, tag="aThi")
            nc.vector.tensor_copy(aThi[:], aThip[:])

            for wb in range(GRPB // 16):
                wbank = wps.tile([128, 512], F32, tag="w")
                for k in range(16):
                    bg = wb * 16 + k
                    b = gb0 + bg
                    ci, bi = b // CH, b % CH
                    cg, fs = k % 4, k // 4
                    oap = wbank[32 * cg:32 * cg + 32, 128 * fs:128 * fs + 128]
                    nc.tensor.matmul(
                        oap, lhsT=aTlo[:, bg:bg + 1].to_broadcast((128, 32)),
                        rhs=knat1_tiles[ci][:, bi, :], start=True, stop=False,
                        tile_position=(0, 32 * cg))
                    nc.tensor.matmul(
                        oap, lhsT=aThi[0:72, bg:bg + 1].to_broadcast((72, 32)),
                        rhs=knat2_tiles[ci][0:72, bi, :], start=False, stop=True,
                        tile_position=(0, 32 * cg))
                wsb = wout_p.tile([128, 512], F32, tag="wsb")
                nc.vector.tensor_copy(wsb[:], wbank[:])
                src = bass.AP(tensor=wsb[:].tensor, offset=wsb[:].offset,
                              ap=[[32 * 512, 4], [128, 4], [1, 128]])
                dst = d["wsum"].rearrange("(o fs cg) d -> o cg fs d", fs=4, cg=4)[
                    (gb0 + wb * 16) // 16]
                nc.sync.dma_start(dst, src)


_BUILD_CACHE = {}


def _get_nc(key, *args, **kwargs):
    if key not in _BUILD_CACHE:
        _BUILD_CACHE[key] = build_nc(*args, **kwargs)
    return _BUILD_CACHE[key]


def host_prep(query, keys, keys_mask, W1, b1, a1, W2, b2, a2, W3, b3, decay_rate):
    """Host-side weight folding. Returns (shared_inputs, meta)."""
    f32 = np.float32
    bf16 = ml_dtypes.bfloat16
    W1 = np.asarray(W1, f32)
    W2 = np.asarray(W2, f32)
    W3 = np.asarray(W3, f32)
    b1 = np.asarray(b1, f32).reshape(H1)
    b2 = np.asarray(b2, f32).reshape(H2)
    b3a = np.asarray(b3, f32).reshape(1)
    a1f = float(np.asarray(a1))
    a2f = float(np.asarray(a2))
    decay = float(np.asarray(decay_rate))

    W1a, W1b, W1c, W1d = W1[0:D], W1[D:2 * D], W1[2 * D:3 * D], W1[3 * D:4 * D]
    Wc1 = (W1a + W1d).astype(bf16)
    Wbd = (W1b - W1d).astype(bf16)
    W1cb = W1c.astype(bf16)

    w3 = W3[:, 0]
    absW3 = np.abs(w3)
    order = np.argsort(w3 < 0, kind="stable")      # W3>=0 columns first
    nP = int((w3 >= 0).sum())
    W2w = ((W2 * absW3[None, :])[:, order]).astype(bf16)
    b2w = ((b2 * absW3)[order]).astype(f32)
    W2eo = np.zeros((2 * H1, 2 * H2), bf16)
    W2eo[0:H1, 0:H2] = W2w
    W2eo[H1:2 * H1, H2:2 * H2] = W2w
    b2wx = np.concatenate([b2w, b2w]).astype(f32)
    b2_zero = not np.any(b2)

    tw = np.exp(decay * (np.arange(L, dtype=np.float64) - L + 1)).astype(f32)
    ident = np.eye(128, dtype=f32)

    shared = {
        "Wc1": Wc1, "W1c": W1cb, "Wbd": Wbd, "W2eo": W2eo,
        "b1": b1, "b2wx": b2wx, "b3": b3a, "tw": tw,
        "idf": ident, "idb": ident.astype(bf16),
    }
    return shared, (a1f, a2f, nP, b2_zero)


def kernel(query, keys, keys_mask, W1, b1, a1, W2, b2, a2, W3, b3, decay_rate,
           trace=False):
    query = np.ascontiguousarray(np.asarray(query, np.float32))
    keys = np.ascontiguousarray(np.asarray(keys, np.float32))
    mask_u8 = np.ascontiguousarray(np.asarray(keys_mask).astype(np.uint8))

    shared, (a1f, a2f, nP, b2_zero) = host_prep(
        query, keys, keys_mask, W1, b1, a1, W2, b2, a2, W3, b3, decay_rate)

    BL = B // NCORES
    key = ("v1", BL, a1f, a2f, nP, b2_zero)
    nc = _get_nc(key, BL, a1f, a2f, nP, b2_zero)

    in_maps = []
    for i in range(NCORES):
        s = slice(i * BL, (i + 1) * BL)
        m = {"query": query[s], "keys": keys[s], "mask": mask_u8[s]}
        m.update(shared)
        in_maps.append(m)

    res = run_bass_kernel_spmd(nc, in_maps, core_ids=list(range(NCORES)),
                               trace=trace)
    ws = np.concatenate([np.asarray(r["wsum"]) for r in res.results], axis=0)
    at = np.concatenate([np.asarray(r["attn"]) for r in res.results], axis=0)
    if trace:
        kernel.last_exec_time_ns = res.exec_time_ns
        kernel.last_results = res
    return ws, at


kernel.last_exec_time_ns = None


# revision 15
# speedup vs baseline: 1.5517x; 1.5517x over previous
"""AdaptiveTimeDecayAttention Trainium2 kernel (8 NeuronCores, pure data parallel).

Math (per batch row b):
    x = [keys, q, keys*q, keys-q]            [L, 4D]
    h1 = prelu(x @ W1 + b1, a1)              [L, H1]
    h2 = prelu(h1 @ W2 + b2, a2)             [L, H2]
    s  = (h2 @ W3 + b3) * tw;  mask;  attn = softmax(s);  ws = attn @ keys

Device decomposition (bf16 matmuls, fp32 accumulation):
    x @ W1 = keys @ (W1a + W1d) + keys @ (diag(q_b) @ W1c) + q_b @ (W1b - W1d)
    (W1 row blocks a/b/c/d).  The diag-folded weight Wt_b = q_b[:,None]*W1c is a
    per-batch [D, H1] stationary.  The q-only term + b1 becomes a per-partition
    bias applied inside the PReLU activation.
    Layer 2+3 fold: scores = sum_j sign(W3_j) * prelu(h1 @ (W2 * |W3|)_j + b2*|W3|_j, a2)
    evaluated in token-on-partition layout so the j-reduction runs on the free axis.
"""

import os
import sys

for _p in ("/root/.axon_site/_ro/trn_rl_repo", "/opt/trn_rl_repo"):
    if os.path.isdir(_p) and _p not in sys.path:
        sys.path.append(_p)

import numpy as np
import ml_dtypes

import concourse.bass as bass
import concourse.tile as tile
from concourse import bacc, mybir
from concourse.bass_utils import run_bass_kernel_spmd

F32 = mybir.dt.float32
BF16 = mybir.dt.bfloat16
U8 = mybir.dt.uint8
AF = mybir.ActivationFunctionType
ALU = mybir.AluOpType

B, L, D = 2048, 200, 128
H1, H2 = 64, 32
NCORES = 8
CH = 16            # batches per chunk
PAIRS = CH // 2


def build_nc(BL, a1f, a2f, nP, b2_zero, b3_zero=True, num_devices=NCORES):
    """Build the per-core Bass program for BL local batches."""
    assert BL % CH == 0
    NCHUNK = BL // CH
    GRPB = min(128, BL)          # batches per softmax group
    assert GRPB % CH == 0
    GRPCH = GRPB // CH
    NGRP = BL // GRPB
    nN = H2 - nP

    nc = bacc.Bacc("TRN2", target_bir_lowering=False, debug=False,
                   num_devices=num_devices)

    def din(name, shape, dt):
        return nc.dram_tensor(name, list(shape), dt, kind="ExternalInput").ap()

    keys_d = din("keys", (BL, L, D), F32)
    query_d = din("query", (BL, D), F32)
    mask_d = din("mask", (BL, L), U8)
    wc1_d = din("Wc1", (D, H1), BF16)
    w1c_d = din("W1c", (D, H1), BF16)
    wbd_d = din("Wbd", (D, H1), BF16)
    w2eo_d = din("W2eo", (2 * H1, 2 * H2), BF16)
    b1_d = din("b1", (H1,), F32)
    b2wx_d = din("b2wx", (2 * H2,), F32)
    b3_d = din("b3", (1,), F32)
    tw_d = din("tw", (L,), F32)
    idf_d = din("idf", (128, 128), F32)
    idb_d = din("idb", (128, 128), BF16)

    wsum_d = nc.dram_tensor("wsum", [BL, D], F32, kind="ExternalOutput").ap()
    attn_d = nc.dram_tensor("attn", [BL, L], F32, kind="ExternalOutput").ap()
    sb2d = nc.dram_tensor("sbounce", [BL, L], F32).ap()

    with tile.TileContext(nc) as tc:
        _kernel_body(tc, dict(
            keys=keys_d, query=query_d, mask=mask_d, Wc1=wc1_d, W1c=w1c_d,
            Wbd=wbd_d, W2eo=w2eo_d, b1=b1_d, b2wx=b2wx_d, b3=b3_d, tw=tw_d,
            idf=idf_d, idb=idb_d, wsum=wsum_d, attn=attn_d, sb2d=sb2d,
        ), BL, NCHUNK, GRPB, GRPCH, NGRP, a1f, a2f, nP, nN, b2_zero, b3_zero)

    nc.compile()
    return nc


def _kernel_body(tc, d, BL, NCHUNK, GRPB, GRPCH, NGRP, a1f, a2f, nP, nN, b2_zero, b3_zero):
    nc = tc.nc
    from contextlib import ExitStack
    ctx = ExitStack()
    with ctx:
        singles = ctx.enter_context(tc.tile_pool(name="singles", bufs=1))
        knat1_p = ctx.enter_context(tc.tile_pool(name="knat1", bufs=GRPCH + 2))
        knat2_p = ctx.enter_context(tc.tile_pool(name="knat2", bufs=GRPCH + 2))
        keysT_p = ctx.enter_context(tc.tile_pool(name="keysT", bufs=3))
        h1x_p = ctx.enter_context(tc.tile_pool(name="h1x", bufs=2))
        wt_p = ctx.enter_context(tc.tile_pool(name="wt", bufs=4))
        r_p = ctx.enter_context(tc.tile_pool(name="r", bufs=2))
        red_p = ctx.enter_context(tc.tile_pool(name="red", bufs=4))
        sc_p = ctx.enter_context(tc.tile_pool(name="scores", bufs=2))
        st_p = ctx.enter_context(tc.tile_pool(name="sT", bufs=2))
        soft_p = ctx.enter_context(tc.tile_pool(name="soft", bufs=3))
        small_p = ctx.enter_context(tc.tile_pool(name="small", bufs=6))
        at_p = ctx.enter_context(tc.tile_pool(name="attnT", bufs=2))
        wout_p = ctx.enter_context(tc.tile_pool(name="wout", bufs=2))

        h1ps = ctx.enter_context(tc.tile_pool(name="h1ps", bufs=2, space="PSUM"))
        ups = ctx.enter_context(tc.tile_pool(name="ups", bufs=2, space="PSUM"))
        mps = ctx.enter_context(tc.tile_pool(name="mps", bufs=2, space="PSUM"))
        wps = ctx.enter_context(tc.tile_pool(name="wps", bufs=2, space="PSUM"))

        # ---------------- setup ----------------
        Wc1s = singles.tile([D, H1], BF16)
        nc.sync.dma_start(Wc1s[:], d["Wc1"])
        W1cs = singles.tile([D, H1], BF16)
        nc.sync.dma_start(W1cs[:], d["W1c"])
        Wbds = singles.tile([D, H1], BF16)
        nc.sync.dma_start(Wbds[:], d["Wbd"])
        W2s = singles.tile([2 * H1, 2 * H2], BF16)
        nc.sync.dma_start(W2s[:], d["W2eo"])
        idfs = singles.tile([128, 128], F32)
        nc.sync.dma_start(idfs[:], d["idf"])
        idbs = singles.tile([128, 128], BF16)
        nc.sync.dma_start(idbs[:], d["idb"])
        b1s = singles.tile([H1, 1], F32)
        nc.sync.dma_start(b1s[:], d["b1"])
        b3s = singles.tile([128, 1], F32)
        nc.sync.dma_start(b3s[:], bass.AP(tensor=d["b3"].tensor, offset=d["b3"].offset,
                                          ap=[[0, 128]] + list(d["b3"].ap)))
        tws = singles.tile([128, L], F32)
        nc.sync.dma_start(tws[:], bass.AP(tensor=d["tw"].tensor, offset=d["tw"].offset,
                                          ap=[[0, 128]] + list(d["tw"].ap)))
        b2wxs = singles.tile([128, 2 * H2], F32)
        nc.sync.dma_start(b2wxs[:], bass.AP(tensor=d["b2wx"].tensor, offset=d["b2wx"].offset,
                                            ap=[[0, 128]] + list(d["b2wx"].ap)))
        negbig = singles.tile([128, 1], F32)
        nc.vector.memset(negbig[:], -1e30)

        # q transpose -> qT [D, BL] (f32 + bf16 copies)
        QP = min(128, BL)
        QT = BL // QP
        qnat = singles.tile([QP, QT, D], F32)
        nc.sync.dma_start(qnat[:], d["query"].rearrange("(t p) d -> p t d", p=QP))
        qTps = mps.tile([128, BL], F32, tag="m")
        for t in range(QT):
            nc.tensor.transpose(qTps[:, t * QP:(t + 1) * QP], qnat[:, t, :],
                                idfs[0:QP, 0:QP])
        qTs = singles.tile([D, BL], F32)
        nc.vector.tensor_copy(qTs[:], qTps[:])
        qTb = singles.tile([D, BL], BF16)
        nc.vector.tensor_copy(qTb[:], qTps[:])

        # qW = q @ (W1b - W1d) + b1, stacked per batch-pair:
        # qWstack[0:64, i] = qW[2i] + b1 ; qWstack[64:128, i] = qW[2i+1] + b1
        qWps = mps.tile([H1, BL], F32, tag="m")
        nc.tensor.matmul(qWps[:], lhsT=Wbds[:], rhs=qTb[:])
        qWstack = singles.tile([128, BL // 2], F32)
        qWv = qWps[:].rearrange("h (i two) -> h two i", two=2)
        nc.scalar.activation(qWstack[0:H1, :], qWv[:, 0, :], AF.Identity, bias=b1s[:])
        nc.scalar.activation(qWstack[H1:128, :], qWv[:, 1, :], AF.Identity, bias=b1s[:])

        knat1_tiles = []
        knat2_tiles = []

        for g in range(NGRP):
            # ------------- chunk phase -------------
            for cc in range(GRPCH):
                c = g * GRPCH + cc
                b0 = c * CH
                knat1 = knat1_p.tile([128, CH, D], BF16, tag="knat1")
                nc.gpsimd.dma_start(
                    knat1[:], d["keys"][b0:b0 + CH, 0:128, :].rearrange("b l d -> l b d"))
                knat2 = knat2_p.tile([80, CH, D], BF16, tag="knat2")
                nc.gpsimd.dma_start(
                    knat2[0:72, :, :],
                    d["keys"][b0:b0 + CH, 128:L, :].rearrange("b l d -> l b d"))
                knat1_tiles.append(knat1)
                knat2_tiles.append(knat2)

                keysT1 = keysT_p.tile([128, CH, 128], BF16, tag="keysT1")
                nc.sync.dma_start(out=keysT1[:], in_=knat1[:], transpose=True)
                keysT2 = keysT_p.tile([128, CH, 80], BF16, tag="keysT2")
                nc.sync.dma_start(out=keysT2[:], in_=knat2[:], transpose=True)

                Wtc = wt_p.tile([D, CH, H1], BF16, tag="wt")
                qv = bass.AP(tensor=qTb[:].tensor, offset=qTb[:].offset + b0,
                             ap=[list(qTb[:].ap[0]), [1, CH], [0, H1]])
                wv = bass.AP(tensor=W1cs[:].tensor, offset=W1cs[:].offset,
                             ap=[list(W1cs[:].ap[0]), [0, CH], [1, H1]])
                nc.vector.tensor_tensor(out=Wtc[:], in0=qv, in1=wv, op=ALU.mult)

                h1x = h1x_p.tile([128, PAIRS * L], BF16, tag="h1x")
                for p in range(PAIRS):
                    gpair = c * PAIRS + p
                    h1bank = h1ps.tile([128, L], F32, tag="h1bank")
                    for par in range(2):
                        bi = 2 * p + par
                        b = b0 + bi
                        cp = H1 * par
                        Wt = Wtc[:, bi, :]
                        for (rhs_ap, o0, o1) in (
                                (keysT1[:, bi, :], 0, 128),
                                (keysT2[:, bi, 0:72], 128, L)):
                            nc.tensor.matmul(
                                h1bank[cp:cp + H1, o0:o1], lhsT=Wc1s[:],
                                rhs=rhs_ap, start=True, stop=False,
                                tile_position=(0, cp), skip_group_check=True)
                            nc.tensor.matmul(
                                h1bank[cp:cp + H1, o0:o1], lhsT=Wt,
                                rhs=rhs_ap, start=False, stop=True,
                                tile_position=(0, cp), skip_group_check=True)
                    nc.scalar.activation(
                        h1x[:, p * L:(p + 1) * L], h1bank[:],
                        AF.Prelu, bias=qWstack[:, gpair:gpair + 1],
                        scale=1.0, alpha=a1f)

                scores = sc_p.tile([128, 32], F32, tag="scores")
                for half in range(2):
                    ubank = ups.tile([128, 512], F32, tag="ubank")
                    for pl in range(4):
                        p = half * 4 + pl
                        for lh in range(2):
                            sl = pl * 2 + lh
                            l0, l1 = (0, 128) if lh == 0 else (128, L)
                            nc.tensor.matmul(
                                ubank[0:l1 - l0, sl * 64:(sl + 1) * 64],
                                lhsT=h1x[:, p * L + l0: p * L + l1],
                                rhs=W2s[:], start=True, stop=True)
                    if not b2_zero:
                        nc.vector.tensor_add(
                            ubank[:], ubank[:],
                            b2wxs[:].rearrange("p j -> p 1 j").to_broadcast((128, 8, 64)))
                    r = r_p.tile([128, 512], BF16, tag="r")
                    nc.scalar.activation(r[:], ubank[:], AF.Prelu,
                                         bias=0.0, scale=1.0, alpha=a2f)
                    rv = r[:].rearrange("p (s j) -> p s j", s=8)
                    scv = scores[:].rearrange("p (lh pq par) -> p pq lh par",
                                              lh=2, par=2)
                    for par in range(2):
                        out_ap = scv[:, half * 4:(half + 1) * 4, :, par]
                        if nP and nN:
                            tP = red_p.tile([128, 8], F32, tag="red")
                            tN = red_p.tile([128, 8], F32, tag="red")
                            nc.vector.tensor_reduce(
                                out=tP[:], in_=rv[:, :, par * 32:par * 32 + nP],
                                axis=mybir.AxisListType.X, op=ALU.add)
                            nc.vector.tensor_reduce(
                                out=tN[:], in_=rv[:, :, par * 32 + nP:par * 32 + 32],
                                axis=mybir.AxisListType.X, op=ALU.add)
                            nc.vector.tensor_tensor(
                                out=out_ap,
                                in0=tP[:].rearrange("p (pl lh) -> p pl lh", lh=2),
                                in1=tN[:].rearrange("p (pl lh) -> p pl lh", lh=2),
                                op=ALU.subtract)
                        else:
                            nc.vector.tensor_reduce(
                                out=out_ap,
                                in_=rv[:, :, par * 32:par * 32 + 32],
                                axis=mybir.AxisListType.X,
                                op=ALU.add, negate=(nN > 0))

                if not b3_zero:
                    nc.vector.tensor_scalar_add(scores[:], scores[:], b3s[:])
                sTp = mps.tile([32, 128], F32, tag="m")
                nc.tensor.transpose(sTp[:], scores[:], idfs[:])
                sT = st_p.tile([32, 128], F32, tag="sT")
                nc.vector.tensor_copy(sT[:], sTp[:])
                nc.sync.dma_start(d["sb2d"][b0:b0 + CH, 0:128], sT[0:16, :])
                nc.sync.dma_start(d["sb2d"][b0:b0 + CH, 128:L], sT[16:32, 0:72])

            # ------------- group phase -------------
            gb0 = g * GRPB
            ssb = soft_p.tile([GRPB, L], F32, tag="ssb")
            nc.sync.dma_start(ssb[:], d["sb2d"][gb0:gb0 + GRPB, :])
            msb = soft_p.tile([GRPB, L], U8, tag="msb")
            nc.sync.dma_start(msb[:], d["mask"][gb0:gb0 + GRPB, :])
            st = soft_p.tile([GRPB, L], F32, tag="st")
            nc.vector.tensor_mul(st[:], ssb[:], tws[0:GRPB, :])
            sm = soft_p.tile([GRPB, L], F32, tag="sm")
            nc.vector.tensor_copy(sm[:], negbig[0:GRPB, :].to_broadcast((GRPB, L)))
            nc.vector.copy_predicated(sm[:], msb[:], st[:])
            negmax = small_p.tile([GRPB, 1], F32, tag="negmax")
            nc.vector.tensor_reduce(out=negmax[:], in_=sm[:], axis=mybir.AxisListType.X,
                                    op=ALU.max, negate=True)
            psb = soft_p.tile([GRPB, L], F32, tag="psb")
            sumexp = small_p.tile([GRPB, 1], F32, tag="sumexp")
            nc.scalar.activation(psb[:], sm[:], AF.Exp, bias=negmax[:],
                                 scale=1.0, accum_out=sumexp[:])
            rsum = small_p.tile([GRPB, 1], F32, tag="rsum")
            nc.vector.reciprocal(rsum[:], sumexp[:])
            attnf = soft_p.tile([GRPB, L], F32, tag="attnf")
            nc.vector.tensor_scalar_mul(attnf[:], psb[:], rsum[:])
            nc.sync.dma_start(d["attn"][gb0:gb0 + GRPB, :], attnf[:])
            attnb = soft_p.tile([GRPB, L], BF16, tag="attnb")
            nc.vector.tensor_copy(attnb[:], attnf[:])

            aTlop = wps.tile([128, GRPB], BF16, tag="w")
            nc.tensor.transpose(aTlop[:], attnb[:, 0:128], idbs[0:GRPB, 0:GRPB])
            aTlo = at_p.tile([128, GRPB], BF16, tag="aTlo")
            nc.vector.tensor_copy(aTlo[:], aTlop[:])
            aThip = wps.tile([72, GRPB], BF16, tag="w")
            nc.tensor.transpose(aThip[:], attnb[:, 128:L], idbs[0:GRPB, 0:GRPB])
            aThi = at_p.tile([72, GRPB], BF16, tag="aThi")
            nc.vector.tensor_copy(aThi[:], aThip[:])

            for wb in range(GRPB // 16):
                wbank = wps.tile([128, 512], F32, tag="w")
                for k in range(16):
                    bg = wb * 16 + k
                    b = gb0 + bg
                    ci, bi = b // CH, b % CH
                    cg, fs = k % 4, k // 4
                    oap = wbank[32 * cg:32 * cg + 32, 128 * fs:128 * fs + 128]
                    nc.tensor.matmul(
                        oap, lhsT=aTlo[:, bg:bg + 1].to_broadcast((128, 32)),
                        rhs=knat1_tiles[ci][:, bi, :], start=True, stop=False,
                        tile_position=(0, 32 * cg))
                    nc.tensor.matmul(
                        oap, lhsT=aThi[0:72, bg:bg + 1].to_broadcast((72, 32)),
                        rhs=knat2_tiles[ci][0:72, bi, :], start=False, stop=True,
                        tile_position=(0, 32 * cg))
                wsb = wout_p.tile([128, 512], F32, tag="wsb")
                nc.vector.tensor_copy(wsb[:], wbank[:])
                src = bass.AP(tensor=wsb[:].tensor, offset=wsb[:].offset,
                              ap=[[32 * 512, 4], [128, 4], [1, 128]])
                dst = d["wsum"].rearrange("(o fs cg) d -> o cg fs d", fs=4, cg=4)[
                    (gb0 + wb * 16) // 16]
                nc.sync.dma_start(dst, src)


_BUILD_CACHE = {}


def _get_nc(key, *args, **kwargs):
    if key not in _BUILD_CACHE:
        _BUILD_CACHE[key] = build_nc(*args, **kwargs)
    return _BUILD_CACHE[key]


def host_prep(query, keys, keys_mask, W1, b1, a1, W2, b2, a2, W3, b3, decay_rate):
    """Host-side weight folding. Returns (shared_inputs, meta)."""
    f32 = np.float32
    bf16 = ml_dtypes.bfloat16
    W1 = np.asarray(W1, f32)
    W2 = np.asarray(W2, f32)
    W3 = np.asarray(W3, f32)
    b1 = np.asarray(b1, f32).reshape(H1)
    b2 = np.asarray(b2, f32).reshape(H2)
    b3a = np.asarray(b3, f32).reshape(1)
    a1f = float(np.asarray(a1))
    a2f = float(np.asarray(a2))
    decay = float(np.asarray(decay_rate))

    W1a, W1b, W1c, W1d = W1[0:D], W1[D:2 * D], W1[2 * D:3 * D], W1[3 * D:4 * D]
    Wc1 = (W1a + W1d).astype(bf16)
    Wbd = (W1b - W1d).astype(bf16)
    W1cb = W1c.astype(bf16)

    w3 = W3[:, 0]
    absW3 = np.abs(w3)
    order = np.argsort(w3 < 0, kind="stable")      # W3>=0 columns first
    nP = int((w3 >= 0).sum())
    W2w = ((W2 * absW3[None, :])[:, order]).astype(bf16)
    b2w = ((b2 * absW3)[order]).astype(f32)
    W2eo = np.zeros((2 * H1, 2 * H2), bf16)
    W2eo[0:H1, 0:H2] = W2w
    W2eo[H1:2 * H1, H2:2 * H2] = W2w
    b2wx = np.concatenate([b2w, b2w]).astype(f32)
    b2_zero = not np.any(b2)

    tw = np.exp(decay * (np.arange(L, dtype=np.float64) - L + 1)).astype(f32)
    ident = np.eye(128, dtype=f32)

    shared = {
        "Wc1": Wc1, "W1c": W1cb, "Wbd": Wbd, "W2eo": W2eo,
        "b1": b1, "b2wx": b2wx, "b3": b3a, "tw": tw,
        "idf": ident, "idb": ident.astype(bf16),
    }
    return shared, (a1f, a2f, nP, b2_zero, not np.any(b3a))


def kernel(query, keys, keys_mask, W1, b1, a1, W2, b2, a2, W3, b3, decay_rate,
           trace=False):
    query = np.ascontiguousarray(np.asarray(query, np.float32))
    keys = np.ascontiguousarray(np.asarray(keys, np.float32))
    mask_u8 = np.ascontiguousarray(np.asarray(keys_mask).astype(np.uint8))

    shared, (a1f, a2f, nP, b2_zero, b3_zero) = host_prep(
        query, keys, keys_mask, W1, b1, a1, W2, b2, a2, W3, b3, decay_rate)

    BL = B // NCORES
    key = ("v2", BL, a1f, a2f, nP, b2_zero, b3_zero)
    nc = _get_nc(key, BL, a1f, a2f, nP, b2_zero, b3_zero)

    in_maps = []
    for i in range(NCORES):
        s = slice(i * BL, (i + 1) * BL)
        m = {"query": query[s], "keys": keys[s], "mask": mask_u8[s]}
        m.update(shared)
        in_maps.append(m)

    res = run_bass_kernel_spmd(nc, in_maps, core_ids=list(range(NCORES)),
                               trace=trace)
    ws = np.concatenate([np.asarray(r["wsum"]) for r in res.results], axis=0)
    at = np.concatenate([np.asarray(r["attn"]) for r in res.results], axis=0)
    if trace:
        kernel.last_exec_time_ns = res.exec_time_ns
        kernel.last_results = res
    return ws, at


kernel.last_exec_time_ns = None
